# revision 19
# baseline (speedup 1.0000x reference)
"""Bass/Trainium2 kernel for nn_Loss: loss = -sum_i log(predictions[i, targets[i]]).

Strategy: data-parallel over the batch axis across 8 NeuronCores; each core
handles R = B/8 = 32768 rows.  Only one element per row is needed (128 KiB of
the 128 MiB shard), so instead of streaming the whole shard we gather exactly
those elements with an indirect (SWDGE) DMA.  The flat element index
idx[i] = i*V + targets[i] is precomputed on the host as part of laying out the
sharded inputs (the predictions tensor itself is only ever read on-device).

Per core:
  gpsimd: SWDGE-load idx -> [16, 2048] int32 (16 big parallel descriptors, and
          the Pool engine exits the preamble first so this is the earliest
          possible load); then one indirect_dma_start per chunk - descriptor
          generation is the pacer (~0.14 ns/index), so chunks are sized
          [12288, 12288, 6144, 2048] to shrink the last chunk's tail.
  scalar: dummy Ln activation (scale=0, bias=1 -> ln(1), input-independent)
          pulls the ACT Ln table load off the critical path; one Ln per chunk
          with accum_out -> lnacc[:, k]; then issues the result DMA itself
          (scalar is an HWDGE engine - no cross-engine handoff).

Pairing: the gather pairs the n-th index of a chunk with the n-th element of
the chunk's out slice picked[:, c0:c1] in flat (partition-major) order, so the
host emits index m of chunk k for batch row r = p*NJ + c0 + j (p = m//w_k,
j = m%w_k).  Any row<->slot bijection works since everything is summed.

Each core returns [P, nsplit] partial sums of ln(picked); the host sums all
8*P*nsplit values and negates (the unshard step).  The ACT Ln table's ~4e-3
per-element error is far inside the 2e-2 tolerance on the 262k-term sum.

Raw bass (no Tile): this container's walrus rejects instructions with attached
multi-sem waits, so synchronization is explicit standalone wait_ge + then_inc.
"""

import contextlib

import numpy as np

import concourse.bass as bass
import concourse.mybir as mybir
from concourse.bass_utils import run_bass_kernel_spmd

B = 262144
V = 1024
NCORES = 8
R = B // NCORES          # rows per core = 32768
P = 128                  # SBUF partitions
NJ = R // P              # elements per partition = 256
CHUNK_COLS = (96, 96, 48, 16)   # NJ columns per chunk; idx counts 128*w each

F32 = mybir.dt.float32
I32 = mybir.dt.int32
Alu = mybir.AluOpType
AF = mybir.ActivationFunctionType

_nc_cache = {}


def build_nc(chunks=CHUNK_COLS):
    key = tuple(chunks)
    if key in _nc_cache:
        return _nc_cache[key]
    assert sum(chunks) == NJ

    nc = bass.Bass()
    preds = nc.dram_tensor("preds", [R, V], F32, kind="ExternalInput")
    tidx = nc.dram_tensor("tidx", [R], I32, kind="ExternalInput")
    out = nc.dram_tensor("out", [P, len(chunks)], F32, kind="ExternalOutput")

    ctx = contextlib.ExitStack()
    with ctx:
        def sb(name, shape, dtype):
            return ctx.enter_context(nc.sbuf_tensor(name, shape, dtype))

        idx = sb("idx", [P, NJ], I32)
        picked = sb("picked", [P, NJ], F32)
        lnp = sb("lnp", [P, NJ], F32)
        lnacc = sb("lnacc", [P, len(chunks)], F32)
        warm = sb("warm", [P, 1], F32)

        t_sem = ctx.enter_context(nc.semaphore("t_sem"))
        g_sem = ctx.enter_context(nc.semaphore("g_sem"))
        a_sem = ctx.enter_context(nc.semaphore("a_sem"))
        out_sem = ctx.enter_context(nc.semaphore("out_sem"))
        block = ctx.enter_context(nc.Block())

        @block.gpsimd
        def _(gpsimd):
            gpsimd.dma_start(
                out=idx[:], in_=tidx[:].rearrange("(p j) -> p j", p=P)
            ).then_inc(t_sem, 16)
            gpsimd.wait_ge(t_sem, 16)
            c0 = 0
            for w in chunks:
                gpsimd.indirect_dma_start(
                    out=picked[:, c0 : c0 + w],
                    out_offset=None,
                    in_=preds[:, :],
                    in_offset=bass.IndirectOffsetOnAxis(
                        ap=idx[:, c0 : c0 + w], axis=1
                    ),
                ).then_inc(g_sem, 16)
                c0 += w

        @block.scalar
        def _(scalar):
            # ln(0*x + 1) = 0: input-independent; forces the Ln table load now
            scalar.activation(
                out=warm[:], in_=warm[:], func=AF.Ln, bias=1.0, scale=0.0
            )
            c0 = 0
            for k, w in enumerate(chunks):
                scalar.wait_ge(g_sem, 16 * (k + 1))
                scalar.activation(
                    out=lnp[:, c0 : c0 + w],
                    in_=picked[:, c0 : c0 + w],
                    func=AF.Ln,
                    accum_out=lnacc[:, k : k + 1],
                ).then_inc(a_sem, 1)
                c0 += w
            # barrier before the out DMA: the READ_ACCUMULATOR micro-op that
            # writes lnacc completes after the LN's then_inc fires, so a bare
            # self-sem is not enough; a trailing in-order ACT no-op is.
            scalar.copy(out=warm[:], in_=warm[:]).then_inc(a_sem, 1)
            scalar.wait_ge(a_sem, len(chunks) + 1)
            scalar.dma_start(out=out[:], in_=lnacc[:]).then_inc(out_sem, 16)

        @block.sync
        def _(sync):
            sync.wait_ge(out_sem, 16)

    _nc_cache[key] = nc
    return nc


def _make_in_maps(predictions, targets, chunks=CHUNK_COLS):
    predictions = np.ascontiguousarray(predictions, dtype=np.float32)
    targets = np.asarray(targets).astype(np.int32)
    rows = np.arange(R, dtype=np.int32) * V
    in_maps = []
    for c in range(NCORES):
        p_shard = predictions[c * R : (c + 1) * R]
        t_shard = targets[c * R : (c + 1) * R]
        in_maps.append({"preds": p_shard, "tidx": rows + t_shard})
    return in_maps


def _run(predictions, targets, trace=False, chunks=CHUNK_COLS, **kwargs):
    in_maps = _make_in_maps(predictions, targets, chunks=chunks)
    nc = build_nc(chunks=chunks)
    res = run_bass_kernel_spmd(nc, in_maps, list(range(NCORES)), trace=trace, **kwargs)
    acc = np.zeros((), dtype=np.float64)
    for c in range(NCORES):
        acc += np.sum(res.results[c]["out"].astype(np.float64))
    return np.float32(-acc), res


def kernel(predictions, targets):
    total, _ = _run(predictions, targets)
    return total
